# revision 9
# baseline (speedup 1.0000x reference)
"""
CIN (Compressed Interaction Network) kernel for Trainium2, 8 NeuronCores.

Problem (hardcoded):
  x: [4096, 32, 64] fp32; w0: [128, 1024]; b0: [128]; w1: [128, 2048]; b1: [128]
  out: [4096, 192] = concat(relu(y0)[:, 64:], relu(y1)).sum(d)

Design (v2 rewrite):
  - Data parallel over batch: 512 samples/core, tokens t=(b,d), T=32768,
    processed in 16 pairs of 2048 tokens.
  - Layer 0 is fully host-precomputed: the symmetric outer product x(x)x is
    folded to 528 channels (i<=j, weights symmetrized), padded to 768 rows =
    3 DoubleRow fp8 k-tile pairs. z0 (scaled x4) and w0sym (x8) are cast to
    e4m3 on host; the 1/32 descale rides the activation evac's scale.
  - Layer 1 uses f-major channel layout: slot g covers f in {2g, 2g+1},
    partition p -> (f = 2g + p//64, h = p%64). The per-slot broadcast side is
    x (host-known): either DMA'd from HBM (D-modes) or built on the PE with
    one-hot matmuls (P-modes). The fixed side is hidden duplicated 2x.
    z1 = xe * hd elementwise on DVE (bf16, 2x mode) or Pool (fp8 out).
    fp8 slots feed DoubleRow fp8 matmuls (2x PE); bf16 slots plain matmuls.
  - relu folded into Act evacs (per-partition scale/bias APs); d-sums via
    tensor_reduce on Pool/DVE.
"""

import sys

import numpy as np
import ml_dtypes

sys.path.insert(0, "/opt/trn_rl_repo")

B_FULL = 4096
N_CORES = 8
BS = B_FULL // N_CORES  # 512
F = 32
D = 64
T = BS * D  # 32768
PAIR = 2048
NPAIR = T // PAIR  # 16
SPP = PAIR // D  # samples per pair = 32
O = 128
H1 = 64

BF16 = ml_dtypes.bfloat16
FP8 = ml_dtypes.float8_e4m3

WSCALE = 8.0  # weights scaled x8 (avoid e4m3 subnormals)
ZSCALE = 4.0  # z (and hidden copy) scaled x4
DESCALE = 1.0 / (WSCALE * ZSCALE)

# ---- L1 slot configuration -------------------------------------------------
# 16 slots; slot s covers f in {2s, 2s+1}. fp8 slots must come first and be
# even in count (DoubleRow pairs). src: 'D' = xe from HBM, 'P' = xe via PE
# one-hot broadcast. mult: engine for z=xe*hd. evac: engine for P-mode psum
# evacuation.
SLOTS = (
    dict(dt=8, src="D", mult="pool"),
    dict(dt=8, src="D", mult="pool"),
    dict(dt=8, src="D", mult="pool"),
    dict(dt=8, src="D", mult="pool"),
    dict(dt=8, src="D", mult="dve"),
    dict(dt=8, src="D", mult="dve"),
    dict(dt=8, src="P", mult="pool", evac="act"),
    dict(dt=8, src="P", mult="pool", evac="act"),
    dict(dt=16, src="D", mult="dve"),
    dict(dt=16, src="D", mult="dve"),
    dict(dt=16, src="D", mult="dve"),
    dict(dt=16, src="D", mult="pool"),
    dict(dt=16, src="P", mult="dve", evac="act"),
    dict(dt=16, src="P", mult="dve", evac="act"),
    dict(dt=16, src="P", mult="dve", evac="act"),
    dict(dt=16, src="P", mult="pool", evac="act"),
)
N8 = sum(1 for s in SLOTS if s["dt"] == 8)
NDR = N8 // 2
N16 = 16 - N8
ND8 = sum(1 for s in SLOTS if s["dt"] == 8 and s["src"] == "D")
ND16 = sum(1 for s in SLOTS if s["dt"] == 16 and s["src"] == "D")
NP = 16 - ND8 - ND16
assert N8 % 2 == 0 and all(s["dt"] == 8 for s in SLOTS[:N8])

# reduce engines (tensor_reduce axis=X is DVE-only)
RED_D0 = "dve"
RED_Y1 = "dve"

NK0 = 3  # L0 DoubleRow k-tile pairs (768 rows)

_CACHE = {}


def _sym_pairs():
    ps = [(i, j) for i in range(F) for j in range(i, F)]  # 528
    while len(ps) < NK0 * 256:
        ps.append((0, 0))  # padded channels get zero weight
    return ps


def _build_nc():
    import concourse.bass as bass  # noqa: F401
    import concourse.tile as tile
    from concourse import bacc, mybir

    bf16 = mybir.dt.bfloat16
    f8 = mybir.dt.float8e4
    f32 = mybir.dt.float32
    Relu = mybir.ActivationFunctionType.Relu
    Copy = mybir.ActivationFunctionType.Copy
    X = mybir.AxisListType.X
    ADD = mybir.AluOpType.add
    DR = mybir.MatmulPerfMode.DoubleRow

    nc = bacc.Bacc(None, target_bir_lowering=False)

    # ---- dram i/o ----
    z0d = nc.dram_tensor("z0d", [128, NK0, 2, T], f8, kind="ExternalInput")
    w0d = nc.dram_tensor("w0d", [128, NK0, 2, O], f8, kind="ExternalInput")
    xe8d = (
        nc.dram_tensor("xe8d", [ND8, 128, T], f8, kind="ExternalInput")
        if ND8
        else None
    )
    xe16d = (
        nc.dram_tensor("xe16d", [ND16, 128, T], bf16, kind="ExternalInput")
        if ND16
        else None
    )
    xt2d = nc.dram_tensor("xt2d", [64, T], bf16, kind="ExternalInput")
    seld = nc.dram_tensor("seld", [64, max(NP, 1), 128], bf16, kind="ExternalInput")
    w18d = (
        nc.dram_tensor("w18d", [128, NDR, 2, O], f8, kind="ExternalInput")
        if NDR
        else None
    )
    w116d = (
        nc.dram_tensor("w116d", [128, N16, O], bf16, kind="ExternalInput")
        if N16
        else None
    )
    sc0d = nc.dram_tensor("sc0d", [O, 1], f32, kind="ExternalInput")
    bi0d = nc.dram_tensor("bi0d", [O, 1], f32, kind="ExternalInput")
    b1d = nc.dram_tensor("b1d", [O, 1], f32, kind="ExternalInput")
    out0 = nc.dram_tensor("out0", [H1, BS], f32, kind="ExternalOutput")
    out1 = nc.dram_tensor("out1", [O, BS], f32, kind="ExternalOutput")

    with tile.TileContext(nc) as tc:
        with (
            tc.tile_pool(name="singles", bufs=1) as singles,
            tc.tile_pool(name="z0p", bufs=2) as z0pool,
            tc.tile_pool(name="xtp", bufs=2) as xtpool,
            tc.tile_pool(name="xe8p", bufs=8) as xe8pool,
            tc.tile_pool(name="xe16p", bufs=6) as xe16pool,
            tc.tile_pool(name="xePp", bufs=5) as xePpool,
            tc.tile_pool(name="hdp", bufs=2) as hdpool,
            tc.tile_pool(name="z8p", bufs=6) as z8pool,
            tc.tile_pool(name="z16p", bufs=10) as z16pool,
            tc.tile_pool(name="y1sbp", bufs=2) as y1sbpool,
            tc.tile_pool(name="py0", bufs=1, space="PSUM") as py0pool,
            tc.tile_pool(name="py1", bufs=2, space="PSUM") as py1pool,
            tc.tile_pool(name="pbc", bufs=1, space="PSUM") as pbcpool,
        ):
            w0s = singles.tile([128, NK0, 2, O], f8)
            nc.gpsimd.dma_start(out=w0s[:], in_=w0d[:])
            if NDR:
                w18s = singles.tile([128, NDR, 2, O], f8)
                nc.gpsimd.dma_start(out=w18s[:], in_=w18d[:])
            if N16:
                w116s = singles.tile([128, N16, O], bf16)
                nc.gpsimd.dma_start(out=w116s[:], in_=w116d[:])
            sels = singles.tile([64, max(NP, 1), 128], bf16)
            nc.gpsimd.dma_start(out=sels[:], in_=seld[:])
            sc0s = singles.tile([O, 1], f32)
            bi0s = singles.tile([O, 1], f32)
            b1s = singles.tile([O, 1], f32)
            nc.gpsimd.dma_start(out=sc0s[:], in_=sc0d[:])
            nc.gpsimd.dma_start(out=bi0s[:], in_=bi0d[:])
            nc.gpsimd.dma_start(out=b1s[:], in_=b1d[:])
            oaccA = singles.tile([128, BS], f32)  # rows 64:128 = direct0 sums
            oacc1 = singles.tile([O, BS], f32)

            eng = {"pool": nc.gpsimd, "dve": nc.vector}

            for P in range(NPAIR):
                sl = slice(P * PAIR, (P + 1) * PAIR)
                osl = slice(P * SPP, (P + 1) * SPP)

                # ---- input DMAs ----
                z0sb = z0pool.tile([128, NK0, 2, PAIR], f8)
                nc.gpsimd.dma_start(out=z0sb[:], in_=z0d[:, :, :, sl])
                xt2 = xtpool.tile([64, PAIR], bf16)
                nc.gpsimd.dma_start(out=xt2[:], in_=xt2d[:, sl])
                xe_tiles = [None] * 16
                i8 = i16 = 0
                for s, cfg in enumerate(SLOTS):
                    if cfg["src"] == "D":
                        if cfg["dt"] == 8:
                            xe = xe8pool.tile([128, PAIR], f8)
                            nc.gpsimd.dma_start(out=xe[:], in_=xe8d[i8, :, sl])
                            i8 += 1
                        else:
                            xe = xe16pool.tile([128, PAIR], bf16)
                            nc.gpsimd.dma_start(out=xe[:], in_=xe16d[i16, :, sl])
                            i16 += 1
                        xe_tiles[s] = xe

                # ---- layer 0 matmuls: 2 halves x 2 chunks x 3 DR ----
                hd = hdpool.tile([128, PAIR], bf16)
                for h in range(2):
                    y0p = py0pool.tile([128, 1024], f32)
                    for s2 in range(2):
                        cs = slice(h * 1024 + s2 * 512, h * 1024 + (s2 + 1) * 512)
                        ps = slice(s2 * 512, (s2 + 1) * 512)
                        for k in range(NK0):
                            nc.tensor.matmul(
                                y0p[:, ps],
                                w0s[:, k, :, :],
                                z0sb[:, k, :, cs],
                                start=(k == 0),
                                stop=(k == NK0 - 1),
                                perf_mode=DR,
                            )
                    # evac: rows 0:64 -> 4*relu(y0+b0) (hidden), rows 64:128 ->
                    # relu(y0+b0) (direct0). per-partition scale/bias APs.
                    nc.scalar.activation(
                        hd[:, h * 1024 : (h + 1) * 1024],
                        y0p[:],
                        Relu,
                        bias=bi0s[:],
                        scale=sc0s[:],
                    )

                # direct0 d-sums (before rows 64:128 are overwritten by dup)
                eng[RED_D0].tensor_reduce(
                    oaccA[H1:O, osl],
                    hd[H1:O, :].rearrange("p (b d) -> p b d", d=D),
                    axis=X,
                    op=ADD,
                )
                # duplicate hidden rows into partitions 64:128; hdB is a
                # second full copy so DVE and Pool multiplies don't contend
                # on the same SBUF banks.
                hdB = hdpool.tile([128, PAIR], bf16, name="hdB")
                nc.gpsimd.dma_start(out=hd[H1:O, :], in_=hd[0:H1, :])
                nc.gpsimd.dma_start(out=hdB[:], in_=hd[:])

                # ---- P-mode broadcasts on PE ----
                ip = 0
                for s, cfg in enumerate(SLOTS):
                    if cfg["src"] != "P":
                        continue
                    xep = xePpool.tile([128, PAIR], f8 if cfg["dt"] == 8 else bf16)
                    for half in range(2):
                        bcp = pbcpool.tile([128, 1024], f32)
                        for s2 in range(2):
                            cs = slice(half * 1024 + s2 * 512, half * 1024 + (s2 + 1) * 512)
                            ps = slice(s2 * 512, (s2 + 1) * 512)
                            nc.tensor.matmul(
                                bcp[:, ps],
                                sels[:, ip, :],
                                xt2[:, cs],
                                start=True,
                                stop=True,
                            )
                        dsl = slice(half * 1024, (half + 1) * 1024)
                        if cfg["evac"] == "act":
                            nc.scalar.activation(xep[:, dsl], bcp[:], Copy)
                        else:
                            eng[cfg["evac"]].tensor_copy(xep[:, dsl], bcp[:])
                    xe_tiles[s] = xep
                    ip += 1

                # ---- z1 multiplies ----
                z8_tiles = []
                for pi in range(NDR):
                    z8t = z8pool.tile([128, 2, PAIR], f8, name="z8")
                    z8_tiles.append(z8t)
                z16_tiles = []
                for i in range(N16):
                    z16t = z16pool.tile([128, PAIR], bf16, name="z16")
                    z16_tiles.append(z16t)
                for s, cfg in enumerate(SLOTS):
                    e = eng[cfg["mult"]]
                    hsrc = hdB if cfg["mult"] == "pool" else hd
                    if cfg["dt"] == 8:
                        e.tensor_mul(z8_tiles[s // 2][:, s % 2, :], xe_tiles[s][:], hsrc[:])
                    else:
                        e.tensor_mul(z16_tiles[s - N8][:], xe_tiles[s][:], hsrc[:])

                # ---- layer 1 matmuls ----
                y1sb = y1sbpool.tile([128, PAIR], bf16)
                for h in range(2):
                    y1p = py1pool.tile([128, 1024], f32)
                    for s2 in range(2):
                        cs = slice(h * 1024 + s2 * 512, h * 1024 + (s2 + 1) * 512)
                        ps = slice(s2 * 512, (s2 + 1) * 512)
                        for pi in range(NDR):
                            nc.tensor.matmul(
                                y1p[:, ps],
                                w18s[:, pi, :, :],
                                z8_tiles[pi][:, :, cs],
                                start=(pi == 0),
                                stop=False,
                                perf_mode=DR,
                                skip_group_check=True,
                            )
                        for i in range(N16):
                            nc.tensor.matmul(
                                y1p[:, ps],
                                w116s[:, i, :],
                                z16_tiles[i][:, cs],
                                start=(NDR == 0 and i == 0),
                                stop=(i == N16 - 1),
                                skip_group_check=True,
                            )
                    nc.scalar.activation(
                        y1sb[:, h * 1024 : (h + 1) * 1024],
                        y1p[:],
                        Relu,
                        bias=b1s[:],
                        scale=DESCALE,
                    )
                eng[RED_Y1].tensor_reduce(
                    oacc1[:, osl],
                    y1sb[:].rearrange("p (b d) -> p b d", d=D),
                    axis=X,
                    op=ADD,
                )

            nc.gpsimd.dma_start(out=out0[:], in_=oaccA[H1:O, :])
            nc.gpsimd.dma_start(out=out1[:], in_=oacc1[:])

    nc.finalize()
    return nc


def _get_nc():
    if "nc" not in _CACHE:
        _CACHE["nc"] = _build_nc()
    return _CACHE["nc"]


def _host_prep(x, w0, b0, w1, b1):
    """Build per-core input maps. x: [4096, 32, 64] fp32."""
    x = np.asarray(x, dtype=np.float32)
    w0 = np.asarray(w0, dtype=np.float32)
    w1 = np.asarray(w1, dtype=np.float32)
    b0 = np.asarray(b0, dtype=np.float32).reshape(O)
    b1 = np.asarray(b1, dtype=np.float32).reshape(O)

    # ---- shared weight-side tensors ----
    pairs = _sym_pairs()
    I = np.array([p[0] for p in pairs])
    J = np.array([p[1] for p in pairs])
    w0sym = np.zeros((O, NK0 * 256), np.float32)
    for c, (i, j) in enumerate(pairs[:528]):
        w0sym[:, c] = w0[:, i * F + j] + (w0[:, j * F + i] if i != j else 0.0)
    # dram layout [128, NK0, 2, O]: c_lin = (2k+j2)*128 + p
    w0d = np.ascontiguousarray(
        (WSCALE * w0sym).T.reshape(NK0, 2, 128, O).transpose(2, 0, 1, 3)
    ).astype(FP8)

    # L1 slot weights: slot s, partition p -> c_orig = (p%64)*F + (2s + p//64)
    pidx = np.arange(128)
    w1slot = np.zeros((16, 128, O), np.float32)
    for s in range(16):
        c_orig = (pidx % 64) * F + (2 * s + pidx // 64)
        w1slot[s] = (WSCALE * w1[:, c_orig]).T
    if NDR:
        w18d = np.ascontiguousarray(
            w1slot[:N8].reshape(NDR, 2, 128, O).transpose(2, 0, 1, 3)
        ).astype(FP8)
    if N16:
        w116d = np.ascontiguousarray(w1slot[N8:].transpose(1, 0, 2)).astype(BF16)

    # one-hot sels for P slots: sel[k, idx, p] = 1 iff k == f(p) + 32*(p%2)
    selp = np.zeros((64, max(NP, 1), 128), np.float32)
    ip = 0
    for s, cfg in enumerate(SLOTS):
        if cfg["src"] != "P":
            continue
        fidx = 2 * s + pidx // 64
        k = fidx + 32 * (pidx % 2)
        selp[k, ip, pidx] = 1.0
        ip += 1
    selp = selp.astype(BF16)

    # Act evac scale/bias for layer 0
    sc0 = np.full((O, 1), DESCALE, np.float32)
    sc0[:H1] = ZSCALE * DESCALE
    bi0 = b0.reshape(O, 1).copy()
    bi0h = bi0.copy()
    bi0h[:H1] *= ZSCALE
    b1c = b1.reshape(O, 1).copy()

    shared = dict(w0d=w0d, sc0d=sc0, bi0d=bi0h, b1d=b1c, seld=selp)
    if NDR:
        shared["w18d"] = w18d
    if N16:
        shared["w116d"] = w116d

    # ---- per-core x-side tensors ----
    xbf = (
        np.ascontiguousarray(
            x.reshape(N_CORES, BS, F, D).transpose(0, 2, 1, 3)
        )
        .astype(BF16)
        .reshape(N_CORES, F, T)
        .astype(np.float32)
    )  # [cores, 32, T] (bf16 values)

    in_maps = []
    for ci in range(N_CORES):
        xc = xbf[ci]  # [32, T]
        z0lin = (ZSCALE * xc[I] * xc[J]).astype(FP8)  # [768, T]
        z0dc = np.ascontiguousarray(
            z0lin.reshape(NK0, 2, 128, T).transpose(2, 0, 1, 3)
        )
        m = dict(shared)
        m["z0d"] = z0dc
        m["xt2d"] = np.ascontiguousarray(np.tile(xc, (2, 1))).astype(BF16)
        i8 = i16 = 0
        xe8l, xe16l = [], []
        for s, cfg in enumerate(SLOTS):
            if cfg["src"] != "D":
                continue
            rows = xc[2 * s + pidx // 64]  # [128, T]
            if cfg["dt"] == 8:
                xe8l.append(rows.astype(FP8))
            else:
                xe16l.append(rows.astype(BF16))
        if xe8l:
            m["xe8d"] = np.ascontiguousarray(np.stack(xe8l))
        if xe16l:
            m["xe16d"] = np.ascontiguousarray(np.stack(xe16l))
        in_maps.append(m)
    return in_maps


def kernel(cin_inputs, w0, b0, w1, b1, _trace=False):
    from concourse.bass_utils import run_bass_kernel_spmd

    in_maps = _host_prep(cin_inputs, w0, b0, w1, b1)
    nc = _get_nc()
    res = run_bass_kernel_spmd(nc, in_maps, core_ids=list(range(N_CORES)), trace=_trace)
    outs = []
    for r in res.results:
        o = np.concatenate([r["out0"], r["out1"]], axis=0).T  # [BS, 192]
        outs.append(o)
    full = np.concatenate(outs, axis=0).astype(np.float32)
    if _trace:
        return full, res
    return full


# revision 10
# speedup vs baseline: 1.1994x; 1.1994x over previous
"""
CIN (Compressed Interaction Network) kernel for Trainium2, 8 NeuronCores.

Problem (hardcoded):
  x: [4096, 32, 64] fp32; w0: [128, 1024]; b0: [128]; w1: [128, 2048]; b1: [128]
  out: [4096, 192] = concat(relu(y0)[:, 64:], relu(y1)).sum(d)

Design (v3, informed by HW probes):
  - Data parallel over batch: 512 samples/core, tokens t=(b,d), T=32768,
    16 pairs of 2048 tokens.
  - Layer 0 fully host-precomputed: symmetric x(x)x folded to 528 channels,
    padded to 768 rows = 3 DoubleRow fp8 k-pairs. Weights x8 / z x4 scaled
    (e4m3 subnormal avoidance); 1/32 descale folded into Act evac scale.
  - Layer 1 f-major (slot g: f in {2g,2g+1}, p -> (f=2g+p//64, h=p%64)).
    All 16 z-tiles are fp8 (DVE bf16-in fp8-out multiplies run in 2x mode:
    measured 1.2us/tile) feeding DoubleRow fp8 matmuls.
    xe sources: 10 slots DMA'd bf16, 2 slots DMA'd fp8 + Act cast to bf16,
    4 slots built on PE via one-hot matmuls + Act psum evac to bf16.
    GpSimd tensor ops are avoided entirely (slow + poisons DVE).
  - DMA layouts are per-pair contiguous so each partition-row descriptor is
    >= 4KB (DMA is descriptor-rate-bound).
  - relu folded into Act evacs; d-sums via DVE tensor_reduce.
"""

import sys

import numpy as np
import ml_dtypes

sys.path.insert(0, "/opt/trn_rl_repo")

B_FULL = 4096
N_CORES = 8
BS = B_FULL // N_CORES  # 512
F = 32
D = 64
T = BS * D  # 32768
PAIR = 2048
NPAIR = T // PAIR  # 16
SPP = PAIR // D  # samples per pair = 32
O = 128
H1 = 64

BF16 = ml_dtypes.bfloat16
FP8 = ml_dtypes.float8_e4m3

WSCALE = 8.0
ZSCALE = 4.0
DESCALE = 1.0 / (WSCALE * ZSCALE)

# L1 slot sources: slots 0..ND16-1 from bf16 DMA, next NC8 from fp8 DMA +
# Act cast, last NP from PE broadcast + Act evac. All z fp8 -> 8 DR pairs.
ND16 = 10
NC8 = 2
NP = 4
assert ND16 + NC8 + NP == 16
assert ND16 % 2 == 0 and NC8 % 2 == 0 and NP % 2 == 0

NK0 = 3  # L0 DoubleRow k-pairs (768 rows)

_CACHE = {}


def _sym_pairs():
    ps = [(i, j) for i in range(F) for j in range(i, F)]  # 528
    while len(ps) < NK0 * 256:
        ps.append((0, 0))
    return ps


def _build_nc():
    import concourse.tile as tile
    from concourse import bacc, mybir

    bf16 = mybir.dt.bfloat16
    f8 = mybir.dt.float8e4
    f32 = mybir.dt.float32
    Relu = mybir.ActivationFunctionType.Relu
    Copy = mybir.ActivationFunctionType.Copy
    X = mybir.AxisListType.X
    ADD = mybir.AluOpType.add
    DR = mybir.MatmulPerfMode.DoubleRow

    nc = bacc.Bacc(None, target_bir_lowering=False)

    # dram inputs; x-side tensors are per-pair-major for large descriptors
    z0d = nc.dram_tensor("z0d", [NPAIR, 128, NK0, 2, PAIR], f8, kind="ExternalInput")
    w0d = nc.dram_tensor("w0d", [128, NK0, 2, O], f8, kind="ExternalInput")
    xe16d = nc.dram_tensor("xe16d", [NPAIR, 128, ND16, PAIR], bf16, kind="ExternalInput")
    xe8d = nc.dram_tensor("xe8d", [NPAIR, 128, NC8, PAIR], f8, kind="ExternalInput")
    xt2d = nc.dram_tensor("xt2d", [64, T], bf16, kind="ExternalInput")
    seld = nc.dram_tensor("seld", [64, NP, 128], bf16, kind="ExternalInput")
    w18d = nc.dram_tensor("w18d", [128, 8, 2, O], f8, kind="ExternalInput")
    sc0d = nc.dram_tensor("sc0d", [O, 1], f32, kind="ExternalInput")
    bi0d = nc.dram_tensor("bi0d", [O, 1], f32, kind="ExternalInput")
    b1d = nc.dram_tensor("b1d", [O, 1], f32, kind="ExternalInput")
    out0 = nc.dram_tensor("out0", [H1, BS], f32, kind="ExternalOutput")
    out1 = nc.dram_tensor("out1", [O, BS], f32, kind="ExternalOutput")

    with tile.TileContext(nc) as tc:
        with (
            tc.tile_pool(name="singles", bufs=1) as singles,
            tc.tile_pool(name="z0p", bufs=2) as z0pool,
            tc.tile_pool(name="xtp", bufs=2) as xtpool,
            tc.tile_pool(name="xe16p", bufs=2) as xe16pool,
            tc.tile_pool(name="xe8p", bufs=2) as xe8pool,
            tc.tile_pool(name="xcp", bufs=3) as xcpool,
            tc.tile_pool(name="hdp", bufs=2) as hdpool,
            tc.tile_pool(name="z8p", bufs=10) as z8pool,
            tc.tile_pool(name="y1sbp", bufs=2) as y1sbpool,
            tc.tile_pool(name="py0", bufs=1, space="PSUM") as py0pool,
            tc.tile_pool(name="py1", bufs=2, space="PSUM") as py1pool,
            tc.tile_pool(name="pbc", bufs=1, space="PSUM") as pbcpool,
        ):
            w0s = singles.tile([128, NK0, 2, O], f8)
            nc.gpsimd.dma_start(out=w0s[:], in_=w0d[:])
            w18s = singles.tile([128, 8, 2, O], f8)
            nc.gpsimd.dma_start(out=w18s[:], in_=w18d[:])
            sels = singles.tile([64, NP, 128], bf16)
            nc.gpsimd.dma_start(out=sels[:], in_=seld[:])
            sc0s = singles.tile([O, 1], f32)
            bi0s = singles.tile([O, 1], f32)
            b1s = singles.tile([O, 1], f32)
            nc.gpsimd.dma_start(out=sc0s[:], in_=sc0d[:])
            nc.gpsimd.dma_start(out=bi0s[:], in_=bi0d[:])
            nc.gpsimd.dma_start(out=b1s[:], in_=b1d[:])
            oaccA = singles.tile([128, BS], f32)
            oacc1 = singles.tile([O, BS], f32)

            for P in range(NPAIR):
                sl = slice(P * PAIR, (P + 1) * PAIR)
                osl = slice(P * SPP, (P + 1) * SPP)

                # ---- input DMAs (one big transfer each) ----
                z0sb = z0pool.tile([128, NK0, 2, PAIR], f8)
                nc.gpsimd.dma_start(out=z0sb[:], in_=z0d[P])
                xe16 = xe16pool.tile([128, ND16, PAIR], bf16)
                nc.gpsimd.dma_start(out=xe16[:], in_=xe16d[P])
                xe8 = xe8pool.tile([128, NC8, PAIR], f8)
                nc.gpsimd.dma_start(out=xe8[:], in_=xe8d[P])
                xt2 = xtpool.tile([64, PAIR], bf16)
                nc.gpsimd.dma_start(out=xt2[:], in_=xt2d[:, sl])

                # ---- layer 0 matmuls ----
                hd = hdpool.tile([128, PAIR], bf16)
                for h in range(2):
                    y0p = py0pool.tile([128, 1024], f32)
                    for s2 in range(2):
                        cs = slice(h * 1024 + s2 * 512, h * 1024 + (s2 + 1) * 512)
                        ps = slice(s2 * 512, (s2 + 1) * 512)
                        for k in range(NK0):
                            nc.tensor.matmul(
                                y0p[:, ps],
                                w0s[:, k, :, :],
                                z0sb[:, k, :, cs],
                                start=(k == 0),
                                stop=(k == NK0 - 1),
                                perf_mode=DR,
                            )
                    # rows 0:64 -> 4*relu(y0+b0) (hidden), 64:128 -> relu (direct0)
                    nc.scalar.activation(
                        hd[:, h * 1024 : (h + 1) * 1024],
                        y0p[:],
                        Relu,
                        bias=bi0s[:],
                        scale=sc0s[:],
                    )

                # direct0 d-sums, then duplicate hidden into rows 64:128
                nc.vector.tensor_reduce(
                    oaccA[H1:O, osl],
                    hd[H1:O, :].rearrange("p (b d) -> p b d", d=D),
                    axis=X,
                    op=ADD,
                )
                nc.gpsimd.dma_start(out=hd[H1:O, :], in_=hd[0:H1, :])

                # ---- xe for C-slots (Act cast fp8->bf16) and P-slots (PE bc) ----
                xe_view = [None] * 16
                for i in range(ND16):
                    xe_view[i] = xe16[:, i, :]
                for i in range(NC8):
                    xc = xcpool.tile([128, PAIR], bf16, name="xc")
                    nc.scalar.activation(xc[:], xe8[:, i, :], Copy)
                    xe_view[ND16 + i] = xc[:]
                for i in range(NP):
                    xp = xcpool.tile([128, PAIR], bf16, name="xp")
                    for half in range(2):
                        bcp = pbcpool.tile([128, 1024], f32)
                        for s2 in range(2):
                            cs = slice(half * 1024 + s2 * 512, half * 1024 + (s2 + 1) * 512)
                            ps = slice(s2 * 512, (s2 + 1) * 512)
                            nc.tensor.matmul(
                                bcp[:, ps], sels[:, i, :], xt2[:, cs],
                                start=True, stop=True,
                            )
                        dsl = slice(half * 1024, (half + 1) * 1024)
                        nc.scalar.activation(xp[:, dsl], bcp[:], Copy)
                    xe_view[ND16 + NC8 + i] = xp[:]

                # ---- z multiplies: all DVE, bf16 x bf16 -> fp8 ----
                z8_tiles = []
                for pi in range(8):
                    z8t = z8pool.tile([128, 2, PAIR], f8, name="z8")
                    z8_tiles.append(z8t)
                for s in range(16):
                    nc.vector.tensor_mul(
                        z8_tiles[s // 2][:, s % 2, :], xe_view[s], hd[:]
                    )

                # ---- layer 1 matmuls: 8 DR pairs ----
                y1sb = y1sbpool.tile([128, PAIR], bf16)
                for h in range(2):
                    y1p = py1pool.tile([128, 1024], f32)
                    for s2 in range(2):
                        cs = slice(h * 1024 + s2 * 512, h * 1024 + (s2 + 1) * 512)
                        ps = slice(s2 * 512, (s2 + 1) * 512)
                        for pi in range(8):
                            nc.tensor.matmul(
                                y1p[:, ps],
                                w18s[:, pi, :, :],
                                z8_tiles[pi][:, :, cs],
                                start=(pi == 0),
                                stop=(pi == 7),
                                perf_mode=DR,
                            )
                    nc.scalar.activation(
                        y1sb[:, h * 1024 : (h + 1) * 1024],
                        y1p[:],
                        Relu,
                        bias=b1s[:],
                        scale=DESCALE,
                    )
                nc.vector.tensor_reduce(
                    oacc1[:, osl],
                    y1sb[:].rearrange("p (b d) -> p b d", d=D),
                    axis=X,
                    op=ADD,
                )

            nc.gpsimd.dma_start(out=out0[:], in_=oaccA[H1:O, :])
            nc.gpsimd.dma_start(out=out1[:], in_=oacc1[:])

    nc.finalize()
    return nc


def _get_nc():
    if "nc" not in _CACHE:
        _CACHE["nc"] = _build_nc()
    return _CACHE["nc"]


def _host_prep(x, w0, b0, w1, b1):
    x = np.asarray(x, dtype=np.float32)
    w0 = np.asarray(w0, dtype=np.float32)
    w1 = np.asarray(w1, dtype=np.float32)
    b0 = np.asarray(b0, dtype=np.float32).reshape(O)
    b1 = np.asarray(b1, dtype=np.float32).reshape(O)

    pairs = _sym_pairs()
    I = np.array([p[0] for p in pairs])
    J = np.array([p[1] for p in pairs])
    w0sym = np.zeros((O, NK0 * 256), np.float32)
    for c, (i, j) in enumerate(pairs[:528]):
        w0sym[:, c] = w0[:, i * F + j] + (w0[:, j * F + i] if i != j else 0.0)
    w0d = np.ascontiguousarray(
        (WSCALE * w0sym).T.reshape(NK0, 2, 128, O).transpose(2, 0, 1, 3)
    ).astype(FP8)

    pidx = np.arange(128)
    w1slot = np.zeros((16, 128, O), np.float32)
    for s in range(16):
        c_orig = (pidx % 64) * F + (2 * s + pidx // 64)
        w1slot[s] = (WSCALE * w1[:, c_orig]).T
    w18d = np.ascontiguousarray(
        w1slot.reshape(8, 2, 128, O).transpose(2, 0, 1, 3)
    ).astype(FP8)

    selp = np.zeros((64, NP, 128), np.float32)
    for i in range(NP):
        s = ND16 + NC8 + i
        fidx = 2 * s + pidx // 64
        k = fidx + 32 * (pidx % 2)
        selp[k, i, pidx] = 1.0
    selp = selp.astype(BF16)

    sc0 = np.full((O, 1), DESCALE, np.float32)
    sc0[:H1] = ZSCALE * DESCALE
    bi0 = b0.reshape(O, 1).copy()
    bi0[:H1] *= ZSCALE
    b1c = b1.reshape(O, 1).copy()

    shared = dict(
        w0d=w0d, w18d=w18d, seld=selp, sc0d=sc0, bi0d=bi0, b1d=b1c
    )

    xbf = (
        np.ascontiguousarray(x.reshape(N_CORES, BS, F, D).transpose(0, 2, 1, 3))
        .astype(BF16)
        .reshape(N_CORES, F, T)
        .astype(np.float32)
    )

    in_maps = []
    for ci in range(N_CORES):
        xc = xbf[ci]  # [32, T] bf16 values
        z0lin = (ZSCALE * xc[I] * xc[J]).astype(FP8)  # [768, T]
        # [768, T] -> [NPAIR, 128, NK0, 2, PAIR]; c_lin = (2k+j)*128 + p
        z0dc = np.ascontiguousarray(
            z0lin.reshape(NK0, 2, 128, NPAIR, PAIR).transpose(3, 2, 0, 1, 4)
        )
        rows = xc[(2 * np.arange(16))[:, None] + (pidx // 64)[None, :]]  # [16,128,T]
        xe16 = np.ascontiguousarray(
            rows[:ND16].reshape(ND16, 128, NPAIR, PAIR).transpose(2, 1, 0, 3)
        ).astype(BF16)
        xe8 = np.ascontiguousarray(
            rows[ND16 : ND16 + NC8].reshape(NC8, 128, NPAIR, PAIR).transpose(2, 1, 0, 3)
        ).astype(FP8)
        m = dict(shared)
        m["z0d"] = z0dc
        m["xe16d"] = xe16
        m["xe8d"] = xe8
        m["xt2d"] = np.ascontiguousarray(np.tile(xc, (2, 1))).astype(BF16)
        in_maps.append(m)
    return in_maps


def kernel(cin_inputs, w0, b0, w1, b1, _trace=False):
    from concourse.bass_utils import run_bass_kernel_spmd

    in_maps = _host_prep(cin_inputs, w0, b0, w1, b1)
    nc = _get_nc()
    res = run_bass_kernel_spmd(nc, in_maps, core_ids=list(range(N_CORES)), trace=_trace)
    outs = []
    for r in res.results:
        o = np.concatenate([r["out0"], r["out1"]], axis=0).T
        outs.append(o)
    full = np.concatenate(outs, axis=0).astype(np.float32)
    if _trace:
        return full, res
    return full


# revision 14
# speedup vs baseline: 1.8169x; 1.5149x over previous
"""
CIN (Compressed Interaction Network) kernel for Trainium2, 8 NeuronCores.

Problem (hardcoded):
  x: [4096, 32, 64] fp32; w0: [128, 1024]; b0: [128]; w1: [128, 2048]; b1: [128]
  out: [4096, 192] = concat(relu(y0)[:, 64:], relu(y1)).sum(d)

Design (v4, HW-measured rates):
  - Data parallel over batch: 512 samples/core, tokens t=(b,d), T=32768,
    16 pairs of 2048 tokens, software-pipelined across pairs.
  - Layer 0 host-precomputed: symmetric x(x)x -> 528 ch -> 768 rows = 3
    DoubleRow fp8 k-pairs (w x8 / z x4 scaling, descale in Act evac).
  - Layer 1 f-major (slot g: f in {2g,2g+1}, p -> (f=2g+p//64, h=p%64)).
    z tiles are bf16 (DVE 2x multiply, 1.2us/tile; fp8 anywhere in the
    multiply drops DVE to 1x). L1 matmuls bf16. GpSimd tensor ops banned
    (slow, poisons DVE). xe: 10 slots bf16 DMA, 2 slots fp8 DMA + Act cast,
    4 slots PE one-hot broadcast + Act evac.
  - Pipeline: iter P runs L0(P) on PE while DVE multiplies pair P-1 and
    reduces pair P-2; L1(P-1) consumes z slot-by-slot (s-outer, 4 live
    PSUM chunk accumulators) so z tiles die fast.
"""

import sys

import numpy as np
import ml_dtypes

sys.path.insert(0, "/opt/trn_rl_repo")

B_FULL = 4096
N_CORES = 8
BS = B_FULL // N_CORES  # 512
F = 32
D = 64
T = BS * D  # 32768
PAIR = 2048
NPAIR = T // PAIR  # 16
SPP = PAIR // D  # 32
O = 128
H1 = 64

BF16 = ml_dtypes.bfloat16
FP8 = ml_dtypes.float8_e4m3

WSCALE = 8.0
ZSCALE = 4.0
DESCALE = 1.0 / (WSCALE * ZSCALE)

ND16 = 10  # xe slots from bf16 DMA
NC8 = 2    # xe slots from fp8 DMA + Act cast
NP = 4     # xe slots from PE one-hot broadcast
NK0 = 3

_CACHE = {}


def _sym_pairs():
    ps = [(i, j) for i in range(F) for j in range(i, F)]
    while len(ps) < NK0 * 256:
        ps.append((0, 0))
    return ps


def _build_nc():
    import concourse.tile as tile
    from concourse import bacc, mybir

    bf16 = mybir.dt.bfloat16
    f8 = mybir.dt.float8e4
    f32 = mybir.dt.float32
    Relu = mybir.ActivationFunctionType.Relu
    Copy = mybir.ActivationFunctionType.Copy
    X = mybir.AxisListType.X
    ADD = mybir.AluOpType.add
    DR = mybir.MatmulPerfMode.DoubleRow

    nc = bacc.Bacc(None, target_bir_lowering=False)

    z0d = nc.dram_tensor("z0d", [NPAIR, 128, NK0, 2, PAIR], f8, kind="ExternalInput")
    w0d = nc.dram_tensor("w0d", [128, NK0, 2, O], f8, kind="ExternalInput")
    xe16d = nc.dram_tensor("xe16d", [NPAIR, 128, ND16, PAIR], bf16, kind="ExternalInput")
    xe8d = nc.dram_tensor("xe8d", [NPAIR, 128, NC8, PAIR], f8, kind="ExternalInput")
    xt2d = nc.dram_tensor("xt2d", [64, T], bf16, kind="ExternalInput")
    seld = nc.dram_tensor("seld", [64, NP, 128], bf16, kind="ExternalInput")
    w1d = nc.dram_tensor("w1d", [128, 16, O], bf16, kind="ExternalInput")
    sc0d = nc.dram_tensor("sc0d", [O, 1], f32, kind="ExternalInput")
    bi0d = nc.dram_tensor("bi0d", [O, 1], f32, kind="ExternalInput")
    b1d = nc.dram_tensor("b1d", [O, 1], f32, kind="ExternalInput")
    out0 = nc.dram_tensor("out0", [H1, BS], f32, kind="ExternalOutput")
    out1 = nc.dram_tensor("out1", [O, BS], f32, kind="ExternalOutput")

    with tile.TileContext(nc) as tc:
        with (
            tc.tile_pool(name="singles", bufs=1) as singles,
            tc.tile_pool(name="z0p", bufs=2) as z0pool,
            tc.tile_pool(name="xtp", bufs=2) as xtpool,
            tc.tile_pool(name="xe16p", bufs=2) as xe16pool,
            tc.tile_pool(name="xe8p", bufs=2) as xe8pool,
            tc.tile_pool(name="xcp", bufs=6) as xcpool,
            tc.tile_pool(name="hdp", bufs=2) as hdpool,
            tc.tile_pool(name="zp", bufs=3) as zpool,
            tc.tile_pool(name="y1sbp", bufs=2) as y1sbpool,
            tc.tile_pool(name="py0", bufs=1, space="PSUM") as py0pool,
            tc.tile_pool(name="py1", bufs=4, space="PSUM") as py1pool,
            tc.tile_pool(name="pbc", bufs=1, space="PSUM") as pbcpool,
        ):
            w0s = singles.tile([128, NK0, 2, O], f8)
            nc.gpsimd.dma_start(out=w0s[:], in_=w0d[:])
            w1s = singles.tile([128, 16, O], bf16)
            nc.gpsimd.dma_start(out=w1s[:], in_=w1d[:])
            sels = singles.tile([64, NP, 128], bf16)
            nc.gpsimd.dma_start(out=sels[:], in_=seld[:])
            sc0s = singles.tile([O, 1], f32)
            bi0s = singles.tile([O, 1], f32)
            b1s = singles.tile([O, 1], f32)
            nc.gpsimd.dma_start(out=sc0s[:], in_=sc0d[:])
            nc.gpsimd.dma_start(out=bi0s[:], in_=bi0d[:])
            nc.gpsimd.dma_start(out=b1s[:], in_=b1d[:])
            oaccA = singles.tile([128, BS], f32)
            oacc1 = singles.tile([O, BS], f32)

            # pipeline state carried across iters
            st = {}

            def dma_stage(P):
                z0sb = z0pool.tile([128, NK0, 2, PAIR], f8, name="z0sb")
                nc.gpsimd.dma_start(out=z0sb[:], in_=z0d[P])
                xe16 = xe16pool.tile([128, ND16, PAIR], bf16, name="xe16")
                nc.gpsimd.dma_start(out=xe16[:], in_=xe16d[P])
                xe8 = xe8pool.tile([128, NC8, PAIR], f8, name="xe8")
                nc.gpsimd.dma_start(out=xe8[:], in_=xe8d[P])
                xt2 = xtpool.tile([64, PAIR], bf16, name="xt2")
                nc.gpsimd.dma_start(out=xt2[:], in_=xt2d[:, P * PAIR : (P + 1) * PAIR])
                st[("in", P)] = (z0sb, xe16, xe8, xt2)

            def l0_stage(P):
                z0sb, xe16, xe8, xt2 = st[("in", P)]
                hd = hdpool.tile([128, PAIR], bf16, name="hd")
                for h in range(2):
                    y0p = py0pool.tile([128, 1024], f32, name="y0p")
                    for s2 in range(2):
                        cs = slice(h * 1024 + s2 * 512, h * 1024 + (s2 + 1) * 512)
                        ps = slice(s2 * 512, (s2 + 1) * 512)
                        for k in range(NK0):
                            nc.tensor.matmul(
                                y0p[:, ps], w0s[:, k, :, :], z0sb[:, k, :, cs],
                                start=(k == 0), stop=(k == NK0 - 1), perf_mode=DR,
                            )
                    nc.scalar.activation(
                        hd[:, h * 1024 : (h + 1) * 1024], y0p[:], Relu,
                        bias=bi0s[:], scale=sc0s[:],
                    )
                st[("hd", P)] = hd

            def xe_stage(P):
                # C-slot casts + P-slot broadcasts -> bf16 xe tiles
                z0sb, xe16, xe8, xt2 = st[("in", P)]
                views = [xe16[:, i, :] for i in range(ND16)]
                for i in range(NC8):
                    xc = xcpool.tile([128, PAIR], bf16, name="xc")
                    nc.scalar.activation(xc[:], xe8[:, i, :], Copy)
                    views.append(xc[:])
                for i in range(NP):
                    xp = xcpool.tile([128, PAIR], bf16, name="xp")
                    for half in range(2):
                        bcp = pbcpool.tile([128, 1024], f32, name="bcp")
                        for s2 in range(2):
                            cs = slice(half * 1024 + s2 * 512, half * 1024 + (s2 + 1) * 512)
                            ps = slice(s2 * 512, (s2 + 1) * 512)
                            nc.tensor.matmul(
                                bcp[:, ps], sels[:, i, :], xt2[:, cs],
                                start=True, stop=True,
                            )
                        dsl = slice(half * 1024, (half + 1) * 1024)
                        nc.scalar.activation(xp[:, dsl], bcp[:], Copy)
                    views.append(xp[:])
                st[("xe", P)] = views

            def d0red_stage(P):
                hd = st[("hd", P)]
                nc.vector.tensor_reduce(
                    oaccA[H1:O, P * SPP : (P + 1) * SPP],
                    hd[H1:O, :].rearrange("p (b d) -> p b d", d=D),
                    axis=X, op=ADD,
                )
                nc.gpsimd.dma_start(out=hd[H1:O, :], in_=hd[0:H1, :])

            def mult_l1_stage(P):
                # interleave DVE multiplies with PE L1 matmuls (s-outer)
                hd = st.pop(("hd", P))
                views = st.pop(("xe", P))
                st.pop(("in", P))
                ch = []
                for c in range(4):
                    y1c = py1pool.tile([128, 512], f32, name="y1c")
                    ch.append(y1c)
                for s in range(16):
                    z = zpool.tile([128, PAIR], bf16, name="z")
                    nc.vector.tensor_mul(z[:], views[s], hd[:])
                    for c in range(4):
                        nc.tensor.matmul(
                            ch[c][:], w1s[:, s, :], z[:, c * 512 : (c + 1) * 512],
                            start=(s == 0), stop=(s == 15),
                        )
                y1sb = y1sbpool.tile([128, PAIR], bf16, name="y1sb")
                for c in range(4):
                    nc.scalar.activation(
                        y1sb[:, c * 512 : (c + 1) * 512], ch[c][:], Relu,
                        bias=b1s[:], scale=DESCALE,
                    )
                st[("y1sb", P)] = y1sb

            def y1red_stage(P):
                y1sb = st.pop(("y1sb", P))
                nc.vector.tensor_reduce(
                    oacc1[:, P * SPP : (P + 1) * SPP],
                    y1sb[:].rearrange("p (b d) -> p b d", d=D),
                    axis=X, op=ADD,
                )

            dma_stage(0)
            for P in range(NPAIR + 2):
                if P + 1 < NPAIR:
                    dma_stage(P + 1)
                if P >= 2:
                    y1red_stage(P - 2)
                if P < NPAIR:
                    l0_stage(P)
                if P >= 1 and P - 1 < NPAIR:
                    mult_l1_stage(P - 1)
                if P < NPAIR:
                    xe_stage(P)
                    d0red_stage(P)

            nc.gpsimd.dma_start(out=out0[:], in_=oaccA[H1:O, :])
            nc.gpsimd.dma_start(out=out1[:], in_=oacc1[:])

    nc.finalize()
    return nc


def _get_nc():
    if "nc" not in _CACHE:
        _CACHE["nc"] = _build_nc()
    return _CACHE["nc"]


def _host_prep(x, w0, b0, w1, b1):
    x = np.asarray(x, dtype=np.float32)
    w0 = np.asarray(w0, dtype=np.float32)
    w1 = np.asarray(w1, dtype=np.float32)
    b0 = np.asarray(b0, dtype=np.float32).reshape(O)
    b1 = np.asarray(b1, dtype=np.float32).reshape(O)

    pairs = _sym_pairs()
    I = np.array([p[0] for p in pairs])
    J = np.array([p[1] for p in pairs])
    w0sym = np.zeros((O, NK0 * 256), np.float32)
    for c, (i, j) in enumerate(pairs[:528]):
        w0sym[:, c] = w0[:, i * F + j] + (w0[:, j * F + i] if i != j else 0.0)
    w0d = np.ascontiguousarray(
        (WSCALE * w0sym).T.reshape(NK0, 2, 128, O).transpose(2, 0, 1, 3)
    ).astype(FP8)

    pidx = np.arange(128)
    w1slot = np.zeros((16, 128, O), np.float32)
    for s in range(16):
        c_orig = (pidx % 64) * F + (2 * s + pidx // 64)
        w1slot[s] = (WSCALE * w1[:, c_orig]).T
    w1d = np.ascontiguousarray(w1slot.transpose(1, 0, 2)).astype(BF16)

    selp = np.zeros((64, NP, 128), np.float32)
    for i in range(NP):
        s = ND16 + NC8 + i
        fidx = 2 * s + pidx // 64
        k = fidx + 32 * (pidx % 2)
        selp[k, i, pidx] = 1.0
    selp = selp.astype(BF16)

    sc0 = np.full((O, 1), DESCALE, np.float32)
    sc0[:H1] = ZSCALE * DESCALE
    bi0 = b0.reshape(O, 1).copy()
    bi0[:H1] *= ZSCALE
    b1c = b1.reshape(O, 1).copy()

    shared = dict(w0d=w0d, w1d=w1d, seld=selp, sc0d=sc0, bi0d=bi0, b1d=b1c)

    xbf = (
        np.ascontiguousarray(x.reshape(N_CORES, BS, F, D).transpose(0, 2, 1, 3))
        .astype(BF16)
        .reshape(N_CORES, F, T)
        .astype(np.float32)
    )

    in_maps = []
    for ci in range(N_CORES):
        xc = xbf[ci]
        z0lin = (ZSCALE * xc[I] * xc[J]).astype(FP8)
        z0dc = np.ascontiguousarray(
            z0lin.reshape(NK0, 2, 128, NPAIR, PAIR).transpose(3, 2, 0, 1, 4)
        )
        rows = xc[(2 * np.arange(16))[:, None] + (pidx // 64)[None, :]]
        xe16 = np.ascontiguousarray(
            rows[:ND16].reshape(ND16, 128, NPAIR, PAIR).transpose(2, 1, 0, 3)
        ).astype(BF16)
        xe8 = np.ascontiguousarray(
            rows[ND16 : ND16 + NC8].reshape(NC8, 128, NPAIR, PAIR).transpose(2, 1, 0, 3)
        ).astype(FP8)
        m = dict(shared)
        m["z0d"] = z0dc
        m["xe16d"] = xe16
        m["xe8d"] = xe8
        m["xt2d"] = np.ascontiguousarray(np.tile(xc, (2, 1))).astype(BF16)
        in_maps.append(m)
    return in_maps


def kernel(cin_inputs, w0, b0, w1, b1, _trace=False):
    from concourse.bass_utils import run_bass_kernel_spmd

    in_maps = _host_prep(cin_inputs, w0, b0, w1, b1)
    nc = _get_nc()
    res = run_bass_kernel_spmd(nc, in_maps, core_ids=list(range(N_CORES)), trace=_trace)
    outs = []
    for r in res.results:
        o = np.concatenate([r["out0"], r["out1"]], axis=0).T
        outs.append(o)
    full = np.concatenate(outs, axis=0).astype(np.float32)
    if _trace:
        return full, res
    return full


# revision 19
# speedup vs baseline: 1.9441x; 1.0700x over previous
"""
CIN (Compressed Interaction Network) kernel for Trainium2, 8 NeuronCores.

Problem (hardcoded):
  x: [4096, 32, 64] fp32; w0: [128, 1024]; b0: [128]; w1: [128, 2048]; b1: [128]
  out: [4096, 192] = concat(relu(y0)[:, 64:], relu(y1)).sum(d)

Design (v4, HW-measured rates):
  - Data parallel over batch: 512 samples/core, tokens t=(b,d), T=32768,
    16 pairs of 2048 tokens, software-pipelined across pairs.
  - Layer 0 host-precomputed: symmetric x(x)x -> 528 ch -> 768 rows = 3
    DoubleRow fp8 k-pairs (w x8 / z x4 scaling, descale in Act evac).
  - Layer 1 f-major (slot g: f in {2g,2g+1}, p -> (f=2g+p//64, h=p%64)).
    z tiles are bf16 (DVE 2x multiply, 1.2us/tile; fp8 anywhere in the
    multiply drops DVE to 1x). L1 matmuls bf16. GpSimd tensor ops banned
    (slow, poisons DVE). xe: 10 slots bf16 DMA, 2 slots fp8 DMA + Act cast,
    4 slots PE one-hot broadcast + Act evac.
  - Pipeline: iter P runs L0(P) on PE while DVE multiplies pair P-1 and
    reduces pair P-2; L1(P-1) consumes z slot-by-slot (s-outer, 4 live
    PSUM chunk accumulators) so z tiles die fast.
"""

import sys

import numpy as np
import ml_dtypes

sys.path.insert(0, "/opt/trn_rl_repo")

B_FULL = 4096
N_CORES = 8
BS = B_FULL // N_CORES  # 512
F = 32
D = 64
T = BS * D  # 32768
PAIR = 2048
NPAIR = T // PAIR  # 16
SPP = PAIR // D  # 32
O = 128
H1 = 64

BF16 = ml_dtypes.bfloat16
FP8 = ml_dtypes.float8_e4m3

WSCALE = 8.0
ZSCALE = 4.0
DESCALE = 1.0 / (WSCALE * ZSCALE)

ND16 = 10  # xe slots from bf16 DMA
NC8 = 2    # xe slots from fp8 DMA + Act cast
NP = 4     # xe slots from PE one-hot broadcast
NK0 = 3

_CACHE = {}


def _sym_pairs():
    ps = [(i, j) for i in range(F) for j in range(i, F)]
    while len(ps) < NK0 * 256:
        ps.append((0, 0))
    return ps


def _build_nc():
    import concourse.tile as tile
    from concourse import bacc, mybir

    bf16 = mybir.dt.bfloat16
    f8 = mybir.dt.float8e4
    f32 = mybir.dt.float32
    Relu = mybir.ActivationFunctionType.Relu
    Copy = mybir.ActivationFunctionType.Copy
    X = mybir.AxisListType.X
    ADD = mybir.AluOpType.add
    DR = mybir.MatmulPerfMode.DoubleRow

    nc = bacc.Bacc(None, target_bir_lowering=False)

    z0d = nc.dram_tensor("z0d", [NPAIR, 128, NK0, 2, PAIR], f8, kind="ExternalInput")
    w0d = nc.dram_tensor("w0d", [128, NK0, 2, O], f8, kind="ExternalInput")
    xe16d = nc.dram_tensor("xe16d", [NPAIR, 128, ND16, PAIR], bf16, kind="ExternalInput")
    xe8d = nc.dram_tensor("xe8d", [NPAIR, 128, NC8, PAIR], f8, kind="ExternalInput")
    xt2d = nc.dram_tensor("xt2d", [64, T], bf16, kind="ExternalInput")
    seld = nc.dram_tensor("seld", [64, NP, 128], bf16, kind="ExternalInput")
    w1d = nc.dram_tensor("w1d", [128, 16, O], bf16, kind="ExternalInput")
    sc0d = nc.dram_tensor("sc0d", [O, 1], f32, kind="ExternalInput")
    bi0d = nc.dram_tensor("bi0d", [O, 1], f32, kind="ExternalInput")
    b1d = nc.dram_tensor("b1d", [O, 1], f32, kind="ExternalInput")
    out0 = nc.dram_tensor("out0", [H1, BS], f32, kind="ExternalOutput")
    out1 = nc.dram_tensor("out1", [O, BS], f32, kind="ExternalOutput")

    with tile.TileContext(nc) as tc:
        with (
            tc.tile_pool(name="singles", bufs=1) as singles,
            tc.tile_pool(name="z0p", bufs=2) as z0pool,
            tc.tile_pool(name="xtp", bufs=2) as xtpool,
            tc.tile_pool(name="xe16p", bufs=2) as xe16pool,
            tc.tile_pool(name="xe8p", bufs=2) as xe8pool,
            tc.tile_pool(name="xcp", bufs=3) as xcpool,
            tc.tile_pool(name="hdp", bufs=2) as hdpool,
            tc.tile_pool(name="zp", bufs=2) as zpool,
            tc.tile_pool(name="y1sbp", bufs=1) as y1sbpool,
            tc.tile_pool(name="py0", bufs=1, space="PSUM") as py0pool,
            tc.tile_pool(name="py1", bufs=4, space="PSUM") as py1pool,
            tc.tile_pool(name="pbc", bufs=1, space="PSUM") as pbcpool,
        ):
            w0s = singles.tile([128, NK0, 2, O], f8)
            nc.gpsimd.dma_start(out=w0s[:], in_=w0d[:])
            w1s = singles.tile([128, 16, O], bf16)
            nc.gpsimd.dma_start(out=w1s[:], in_=w1d[:])
            sels = singles.tile([64, NP, 128], bf16)
            nc.gpsimd.dma_start(out=sels[:], in_=seld[:])
            sc0s = singles.tile([O, 1], f32)
            bi0s = singles.tile([O, 1], f32)
            b1s = singles.tile([O, 1], f32)
            nc.gpsimd.dma_start(out=sc0s[:], in_=sc0d[:])
            nc.gpsimd.dma_start(out=bi0s[:], in_=bi0d[:])
            nc.gpsimd.dma_start(out=b1s[:], in_=b1d[:])
            oaccA = singles.tile([128, BS], f32)
            oacc1 = singles.tile([O, BS], f32)

            # pipeline state carried across iters
            st = {}

            def dma_stage(P):
                z0sb = z0pool.tile([128, NK0, 2, PAIR], f8, name="z0sb")
                nc.gpsimd.dma_start(out=z0sb[:], in_=z0d[P])
                xe16 = xe16pool.tile([128, ND16, PAIR], bf16, name="xe16")
                nc.gpsimd.dma_start(out=xe16[:], in_=xe16d[P])
                xe8 = xe8pool.tile([128, NC8, PAIR], f8, name="xe8")
                nc.gpsimd.dma_start(out=xe8[:], in_=xe8d[P])
                xt2 = xtpool.tile([64, PAIR], bf16, name="xt2")
                nc.gpsimd.dma_start(out=xt2[:], in_=xt2d[:, P * PAIR : (P + 1) * PAIR])
                st[("in", P)] = (z0sb, xe16, xe8, xt2)

            def l0_stage(P):
                z0sb, xe16, xe8, xt2 = st[("in", P)]
                hd = hdpool.tile([128, PAIR], bf16, name="hd")
                for h in range(2):
                    y0p = py0pool.tile([128, 1024], f32, name="y0p")
                    for s2 in range(2):
                        cs = slice(h * 1024 + s2 * 512, h * 1024 + (s2 + 1) * 512)
                        ps = slice(s2 * 512, (s2 + 1) * 512)
                        for k in range(NK0):
                            nc.tensor.matmul(
                                y0p[:, ps], w0s[:, k, :, :], z0sb[:, k, :, cs],
                                start=(k == 0), stop=(k == NK0 - 1), perf_mode=DR,
                            )
                    nc.scalar.activation(
                        hd[:, h * 1024 : (h + 1) * 1024], y0p[:], Relu,
                        bias=bi0s[:], scale=sc0s[:],
                    )
                st[("hd", P)] = hd

            def xe_stage(P):
                # C-slot casts + P-slot broadcasts -> bf16 xe PAIR tiles
                z0sb, xe16, xe8, xt2 = st[("in", P)]
                pairs = [xe16[:, 2 * i : 2 * i + 2, :] for i in range(ND16 // 2)]
                xc2 = xcpool.tile([128, 2, PAIR], bf16, name="xc2")
                for i in range(NC8):
                    nc.scalar.activation(xc2[:, i, :], xe8[:, i, :], Copy)
                pairs.append(xc2[:])
                for pp in range(NP // 2):
                    xp2 = xcpool.tile([128, 2, PAIR], bf16, name="xp2")
                    for j in range(2):
                        for half in range(2):
                            bcp = pbcpool.tile([128, 1024], f32, name="bcp")
                            for s2 in range(2):
                                cs = slice(half * 1024 + s2 * 512, half * 1024 + (s2 + 1) * 512)
                                ps = slice(s2 * 512, (s2 + 1) * 512)
                                nc.tensor.matmul(
                                    bcp[:, ps], sels[:, 2 * pp + j, :], xt2[:, cs],
                                    start=True, stop=True,
                                )
                            dsl = slice(half * 1024, (half + 1) * 1024)
                            nc.scalar.activation(xp2[:, j, dsl], bcp[:], Copy)
                    pairs.append(xp2[:])
                st[("xe", P)] = pairs

            def d0red_stage(P):
                hd = st[("hd", P)]
                nc.vector.tensor_reduce(
                    oaccA[H1:O, P * SPP : (P + 1) * SPP],
                    hd[H1:O, :].rearrange("p (b d) -> p b d", d=D),
                    axis=X, op=ADD,
                )
                nc.gpsimd.dma_start(out=hd[H1:O, :], in_=hd[0:H1, :])

            def mult_l1_stage(P):
                # paired DVE multiplies (8 ops) + PE L1 matmuls (slot-outer)
                hd = st.pop(("hd", P))
                pairs = st.pop(("xe", P))
                st.pop(("in", P))
                hdb = hd[:].unsqueeze(1).broadcast_to([128, 2, PAIR])
                ch = []
                for c in range(4):
                    y1c = py1pool.tile([128, 512], f32, name="y1c")
                    ch.append(y1c)
                for pi in range(8):
                    z = zpool.tile([128, 2, PAIR], bf16, name="z")
                    nc.vector.tensor_mul(z[:], pairs[pi], hdb)
                    for j in range(2):
                        s = 2 * pi + j
                        for c in range(4):
                            nc.tensor.matmul(
                                ch[c][:], w1s[:, s, :], z[:, j, c * 512 : (c + 1) * 512],
                                start=(s == 0), stop=(s == 15),
                            )
                y1sb = y1sbpool.tile([128, PAIR], bf16, name="y1sb")
                for c in range(4):
                    nc.scalar.activation(
                        y1sb[:, c * 512 : (c + 1) * 512], ch[c][:], Relu,
                        bias=b1s[:], scale=DESCALE,
                    )
                st[("y1sb", P)] = y1sb

            def y1red_stage(P):
                y1sb = st.pop(("y1sb", P))
                nc.vector.tensor_reduce(
                    oacc1[:, P * SPP : (P + 1) * SPP],
                    y1sb[:].rearrange("p (b d) -> p b d", d=D),
                    axis=X, op=ADD,
                )

            dma_stage(0)
            for P in range(NPAIR + 1):
                if P + 1 < NPAIR:
                    dma_stage(P + 1)
                if P < NPAIR:
                    l0_stage(P)
                    xe_stage(P)
                if P >= 1:
                    mult_l1_stage(P - 1)
                if P < NPAIR:
                    d0red_stage(P)
                if P >= 1:
                    y1red_stage(P - 1)

            nc.gpsimd.dma_start(out=out0[:], in_=oaccA[H1:O, :])
            nc.gpsimd.dma_start(out=out1[:], in_=oacc1[:])

    nc.finalize()
    return nc


def _get_nc():
    if "nc" not in _CACHE:
        _CACHE["nc"] = _build_nc()
    return _CACHE["nc"]


def _host_prep(x, w0, b0, w1, b1):
    x = np.asarray(x, dtype=np.float32)
    w0 = np.asarray(w0, dtype=np.float32)
    w1 = np.asarray(w1, dtype=np.float32)
    b0 = np.asarray(b0, dtype=np.float32).reshape(O)
    b1 = np.asarray(b1, dtype=np.float32).reshape(O)

    pairs = _sym_pairs()
    I = np.array([p[0] for p in pairs])
    J = np.array([p[1] for p in pairs])
    w0sym = np.zeros((O, NK0 * 256), np.float32)
    for c, (i, j) in enumerate(pairs[:528]):
        w0sym[:, c] = w0[:, i * F + j] + (w0[:, j * F + i] if i != j else 0.0)
    w0d = np.ascontiguousarray(
        (WSCALE * w0sym).T.reshape(NK0, 2, 128, O).transpose(2, 0, 1, 3)
    ).astype(FP8)

    pidx = np.arange(128)
    w1slot = np.zeros((16, 128, O), np.float32)
    for s in range(16):
        c_orig = (pidx % 64) * F + (2 * s + pidx // 64)
        w1slot[s] = (WSCALE * w1[:, c_orig]).T
    w1d = np.ascontiguousarray(w1slot.transpose(1, 0, 2)).astype(BF16)

    selp = np.zeros((64, NP, 128), np.float32)
    for i in range(NP):
        s = ND16 + NC8 + i
        fidx = 2 * s + pidx // 64
        k = fidx + 32 * (pidx % 2)
        selp[k, i, pidx] = 1.0
    selp = selp.astype(BF16)

    sc0 = np.full((O, 1), DESCALE, np.float32)
    sc0[:H1] = ZSCALE * DESCALE
    bi0 = b0.reshape(O, 1).copy()
    bi0[:H1] *= ZSCALE
    b1c = b1.reshape(O, 1).copy()

    shared = dict(w0d=w0d, w1d=w1d, seld=selp, sc0d=sc0, bi0d=bi0, b1d=b1c)

    xbf = (
        np.ascontiguousarray(x.reshape(N_CORES, BS, F, D).transpose(0, 2, 1, 3))
        .astype(BF16)
        .reshape(N_CORES, F, T)
        .astype(np.float32)
    )

    in_maps = []
    for ci in range(N_CORES):
        xc = xbf[ci]
        z0lin = (ZSCALE * xc[I] * xc[J]).astype(FP8)
        z0dc = np.ascontiguousarray(
            z0lin.reshape(NK0, 2, 128, NPAIR, PAIR).transpose(3, 2, 0, 1, 4)
        )
        rows = xc[(2 * np.arange(16))[:, None] + (pidx // 64)[None, :]]
        xe16 = np.ascontiguousarray(
            rows[:ND16].reshape(ND16, 128, NPAIR, PAIR).transpose(2, 1, 0, 3)
        ).astype(BF16)
        xe8 = np.ascontiguousarray(
            rows[ND16 : ND16 + NC8].reshape(NC8, 128, NPAIR, PAIR).transpose(2, 1, 0, 3)
        ).astype(FP8)
        m = dict(shared)
        m["z0d"] = z0dc
        m["xe16d"] = xe16
        m["xe8d"] = xe8
        m["xt2d"] = np.ascontiguousarray(np.tile(xc, (2, 1))).astype(BF16)
        in_maps.append(m)
    return in_maps


def kernel(cin_inputs, w0, b0, w1, b1, _trace=False):
    from concourse.bass_utils import run_bass_kernel_spmd

    in_maps = _host_prep(cin_inputs, w0, b0, w1, b1)
    nc = _get_nc()
    res = run_bass_kernel_spmd(nc, in_maps, core_ids=list(range(N_CORES)), trace=_trace)
    outs = []
    for r in res.results:
        o = np.concatenate([r["out0"], r["out1"]], axis=0).T
        outs.append(o)
    full = np.concatenate(outs, axis=0).astype(np.float32)
    if _trace:
        return full, res
    return full


# revision 22
# speedup vs baseline: 2.0250x; 1.0416x over previous
"""
CIN (Compressed Interaction Network) kernel for Trainium2, 8 NeuronCores.

Problem (hardcoded):
  x: [4096, 32, 64] fp32; w0: [128, 1024]; b0: [128]; w1: [128, 2048]; b1: [128]
  out: [4096, 192] = concat(relu(y0)[:, 64:], relu(y1)).sum(d)

Design (v4, HW-measured rates):
  - Data parallel over batch: 512 samples/core, tokens t=(b,d), T=32768,
    16 pairs of 2048 tokens, software-pipelined across pairs.
  - Layer 0 host-precomputed: symmetric x(x)x -> 528 ch -> 768 rows = 3
    DoubleRow fp8 k-pairs (w x8 / z x4 scaling, descale in Act evac).
  - Layer 1 f-major (slot g: f in {2g,2g+1}, p -> (f=2g+p//64, h=p%64)).
    z tiles are bf16 (DVE 2x multiply, 1.2us/tile; fp8 anywhere in the
    multiply drops DVE to 1x). L1 matmuls bf16. GpSimd tensor ops banned
    (slow, poisons DVE). xe: 10 slots bf16 DMA, 2 slots fp8 DMA + Act cast,
    4 slots PE one-hot broadcast + Act evac.
  - Pipeline: iter P runs L0(P) on PE while DVE multiplies pair P-1 and
    reduces pair P-2; L1(P-1) consumes z slot-by-slot (s-outer, 4 live
    PSUM chunk accumulators) so z tiles die fast.
"""

import sys

import numpy as np
import ml_dtypes

sys.path.insert(0, "/opt/trn_rl_repo")

B_FULL = 4096
N_CORES = 8
BS = B_FULL // N_CORES  # 512
F = 32
D = 64
T = BS * D  # 32768
PAIR = 2048
NPAIR = T // PAIR  # 16
SPP = PAIR // D  # 32
O = 128
H1 = 64

BF16 = ml_dtypes.bfloat16
FP8 = ml_dtypes.float8_e4m3

WSCALE = 8.0
ZSCALE = 4.0
DESCALE = 1.0 / (WSCALE * ZSCALE)

ND16 = 10  # xe slots from bf16 DMA
NC8 = 2    # xe slots from fp8 DMA + Act cast
NP = 4     # xe slots from PE one-hot broadcast
NK0 = 3

_CACHE = {}


def _sym_pairs():
    ps = [(i, j) for i in range(F) for j in range(i, F)]
    while len(ps) < NK0 * 256:
        ps.append((0, 0))
    return ps


def _build_nc():
    import concourse.tile as tile
    from concourse import bacc, mybir

    bf16 = mybir.dt.bfloat16
    f8 = mybir.dt.float8e4
    f32 = mybir.dt.float32
    Relu = mybir.ActivationFunctionType.Relu
    Copy = mybir.ActivationFunctionType.Copy
    X = mybir.AxisListType.X
    ADD = mybir.AluOpType.add
    DR = mybir.MatmulPerfMode.DoubleRow

    nc = bacc.Bacc(None, target_bir_lowering=False)

    z0d = nc.dram_tensor("z0d", [NPAIR, 2, 128, NK0, 2, PAIR // 2], f8, kind="ExternalInput")
    w0d = nc.dram_tensor("w0d", [128, NK0, 2, O], f8, kind="ExternalInput")
    xe16d = nc.dram_tensor("xe16d", [NPAIR, 128, ND16, PAIR], bf16, kind="ExternalInput")
    xe8d = nc.dram_tensor("xe8d", [NPAIR, 128, NC8, PAIR], f8, kind="ExternalInput")
    xt2d = nc.dram_tensor("xt2d", [128, T], bf16, kind="ExternalInput")
    seld = nc.dram_tensor("seld", [128, NP, 128], bf16, kind="ExternalInput")
    w1d = nc.dram_tensor("w1d", [128, 16, O], bf16, kind="ExternalInput")
    sc0d = nc.dram_tensor("sc0d", [O, 1], f32, kind="ExternalInput")
    bi0d = nc.dram_tensor("bi0d", [O, 1], f32, kind="ExternalInput")
    b1d = nc.dram_tensor("b1d", [O, 1], f32, kind="ExternalInput")
    out0 = nc.dram_tensor("out0", [H1, BS], f32, kind="ExternalOutput")
    out1 = nc.dram_tensor("out1", [O, BS], f32, kind="ExternalOutput")

    with tile.TileContext(nc) as tc:
        with (
            tc.tile_pool(name="singles", bufs=1) as singles,
            tc.tile_pool(name="z0p", bufs=3) as z0pool,
            tc.tile_pool(name="xtp", bufs=2) as xtpool,
            tc.tile_pool(name="xe16p", bufs=2) as xe16pool,
            tc.tile_pool(name="xe8p", bufs=1) as xe8pool,
            tc.tile_pool(name="xcp", bufs=3) as xcpool,
            tc.tile_pool(name="hdp", bufs=2) as hdpool,
            tc.tile_pool(name="zp", bufs=3) as zpool,
            tc.tile_pool(name="y1sbp", bufs=1) as y1sbpool,
            tc.tile_pool(name="py0", bufs=1, space="PSUM") as py0pool,
            tc.tile_pool(name="py1", bufs=4, space="PSUM") as py1pool,
            tc.tile_pool(name="pbc", bufs=1, space="PSUM") as pbcpool,
        ):
            w0s = singles.tile([128, NK0, 2, O], f8)
            nc.gpsimd.dma_start(out=w0s[:], in_=w0d[:])
            w1s = singles.tile([128, 16, O], bf16)
            nc.gpsimd.dma_start(out=w1s[:], in_=w1d[:])
            sels = singles.tile([128, NP, 128], bf16)
            nc.gpsimd.dma_start(out=sels[:], in_=seld[:])
            sc0s = singles.tile([O, 1], f32)
            bi0s = singles.tile([O, 1], f32)
            b1s = singles.tile([O, 1], f32)
            nc.gpsimd.dma_start(out=sc0s[:], in_=sc0d[:])
            nc.gpsimd.dma_start(out=bi0s[:], in_=bi0d[:])
            nc.gpsimd.dma_start(out=b1s[:], in_=b1d[:])
            oaccA = singles.tile([128, BS], f32)
            oacc1 = singles.tile([O, BS], f32)

            # pipeline state carried across iters
            st = {}

            def dma_stage(P):
                z0sb = []
                for h in range(2):
                    z0h = z0pool.tile([128, NK0, 2, PAIR // 2], f8, name="z0sb")
                    nc.gpsimd.dma_start(out=z0h[:], in_=z0d[P, h])
                    z0sb.append(z0h)
                xe16 = xe16pool.tile([128, ND16, PAIR], bf16, name="xe16")
                nc.gpsimd.dma_start(out=xe16[:], in_=xe16d[P])
                xe8 = xe8pool.tile([128, NC8, PAIR], f8, name="xe8")
                nc.gpsimd.dma_start(out=xe8[:], in_=xe8d[P])
                xt2 = xtpool.tile([128, PAIR], bf16, name="xt2")
                nc.gpsimd.dma_start(out=xt2[:], in_=xt2d[:, P * PAIR : (P + 1) * PAIR])
                st[("in", P)] = (z0sb, xe16, xe8, xt2)

            def l0_stage(P):
                z0sb, xe16, xe8, xt2 = st[("in", P)]
                hd = hdpool.tile([128, PAIR], bf16, name="hd")
                for h in range(2):
                    y0p = py0pool.tile([128, 1024], f32, name="y0p")
                    for s2 in range(2):
                        cs = slice(s2 * 512, (s2 + 1) * 512)
                        ps = slice(s2 * 512, (s2 + 1) * 512)
                        for k in range(NK0):
                            nc.tensor.matmul(
                                y0p[:, ps], w0s[:, k, :, :], z0sb[h][:, k, :, cs],
                                start=(k == 0), stop=(k == NK0 - 1), perf_mode=DR,
                            )
                    nc.scalar.activation(
                        hd[:, h * 1024 : (h + 1) * 1024], y0p[:], Relu,
                        bias=bi0s[:], scale=sc0s[:],
                    )
                st[("hd", P)] = hd

            def xe_stage(P):
                # C-slot casts + P-slot broadcasts -> bf16 xe PAIR tiles
                z0sb, xe16, xe8, xt2 = st[("in", P)]
                pairs = [xe16[:, 2 * i : 2 * i + 2, :] for i in range(ND16 // 2)]
                xc2 = xcpool.tile([128, 2, PAIR], bf16, name="xc2")
                for i in range(NC8):
                    nc.scalar.activation(xc2[:, i, :], xe8[:, i, :], Copy)
                pairs.append(xc2[:])
                for pp in range(NP // 2):
                    xp2 = xcpool.tile([128, 2, PAIR], bf16, name="xp2")
                    for j in range(2):
                        for half in range(2):
                            bcp = pbcpool.tile([128, 1024], f32, name="bcp")
                            for s2 in range(2):
                                cs = slice(half * 1024 + s2 * 512, half * 1024 + (s2 + 1) * 512)
                                ps = slice(s2 * 512, (s2 + 1) * 512)
                                nc.tensor.matmul(
                                    bcp[:, ps], sels[:, 2 * pp + j, :], xt2[:, cs],
                                    start=True, stop=True,
                                )
                            dsl = slice(half * 1024, (half + 1) * 1024)
                            nc.scalar.activation(xp2[:, j, dsl], bcp[:], Copy)
                    pairs.append(xp2[:])
                st[("xe", P)] = pairs

            def d0red_stage(P):
                hd = st[("hd", P)]
                nc.vector.tensor_reduce(
                    oaccA[H1:O, P * SPP : (P + 1) * SPP],
                    hd[H1:O, :].rearrange("p (b d) -> p b d", d=D),
                    axis=X, op=ADD,
                )
                nc.gpsimd.dma_start(out=hd[H1:O, :], in_=hd[0:H1, :])

            def mult_l1_stage(P):
                # paired DVE multiplies (8 ops) + PE L1 matmuls (slot-outer)
                hd = st.pop(("hd", P))
                pairs = st.pop(("xe", P))
                st.pop(("in", P))
                hdb = hd[:].unsqueeze(1).broadcast_to([128, 2, PAIR])
                ch = []
                for c in range(4):
                    y1c = py1pool.tile([128, 512], f32, name="y1c")
                    ch.append(y1c)
                for pi in range(8):
                    z = zpool.tile([128, 2, PAIR], bf16, name="z")
                    nc.vector.tensor_mul(z[:], pairs[pi], hdb)
                    for j in range(2):
                        s = 2 * pi + j
                        for c in range(4):
                            nc.tensor.matmul(
                                ch[c][:], w1s[:, s, :], z[:, j, c * 512 : (c + 1) * 512],
                                start=(s == 0), stop=(s == 15),
                            )
                y1sb = y1sbpool.tile([128, PAIR], bf16, name="y1sb")
                for c in range(4):
                    nc.scalar.activation(
                        y1sb[:, c * 512 : (c + 1) * 512], ch[c][:], Relu,
                        bias=b1s[:], scale=DESCALE,
                    )
                st[("y1sb", P)] = y1sb

            def y1red_stage(P):
                y1sb = st.pop(("y1sb", P))
                nc.vector.tensor_reduce(
                    oacc1[:, P * SPP : (P + 1) * SPP],
                    y1sb[:].rearrange("p (b d) -> p b d", d=D),
                    axis=X, op=ADD,
                )

            dma_stage(0)
            for P in range(NPAIR + 1):
                if P + 1 < NPAIR:
                    dma_stage(P + 1)
                if P < NPAIR:
                    l0_stage(P)
                    xe_stage(P)
                if P >= 1:
                    mult_l1_stage(P - 1)
                if P < NPAIR:
                    d0red_stage(P)
                if P >= 1:
                    y1red_stage(P - 1)

            nc.gpsimd.dma_start(out=out0[:], in_=oaccA[H1:O, :])
            nc.gpsimd.dma_start(out=out1[:], in_=oacc1[:])

    nc.finalize()
    return nc


def _get_nc():
    if "nc" not in _CACHE:
        _CACHE["nc"] = _build_nc()
    return _CACHE["nc"]


def _host_prep(x, w0, b0, w1, b1):
    x = np.asarray(x, dtype=np.float32)
    w0 = np.asarray(w0, dtype=np.float32)
    w1 = np.asarray(w1, dtype=np.float32)
    b0 = np.asarray(b0, dtype=np.float32).reshape(O)
    b1 = np.asarray(b1, dtype=np.float32).reshape(O)

    pairs = _sym_pairs()
    I = np.array([p[0] for p in pairs])
    J = np.array([p[1] for p in pairs])
    w0sym = np.zeros((O, NK0 * 256), np.float32)
    for c, (i, j) in enumerate(pairs[:528]):
        w0sym[:, c] = w0[:, i * F + j] + (w0[:, j * F + i] if i != j else 0.0)
    w0d = np.ascontiguousarray(
        (WSCALE * w0sym).T.reshape(NK0, 2, 128, O).transpose(2, 0, 1, 3)
    ).astype(FP8)

    pidx = np.arange(128)
    w1slot = np.zeros((16, 128, O), np.float32)
    for s in range(16):
        c_orig = (pidx % 64) * F + (2 * s + pidx // 64)
        w1slot[s] = (WSCALE * w1[:, c_orig]).T
    w1d = np.ascontiguousarray(w1slot.transpose(1, 0, 2)).astype(BF16)

    selp = np.zeros((128, NP, 128), np.float32)
    for i in range(NP):
        s = ND16 + NC8 + i
        fidx = 2 * s + pidx // 64
        k = fidx + 32 * (pidx % 4)
        selp[k, i, pidx] = 1.0
    selp = selp.astype(BF16)

    sc0 = np.full((O, 1), DESCALE, np.float32)
    sc0[:H1] = ZSCALE * DESCALE
    bi0 = b0.reshape(O, 1).copy()
    bi0[:H1] *= ZSCALE
    b1c = b1.reshape(O, 1).copy()

    shared = dict(w0d=w0d, w1d=w1d, seld=selp, sc0d=sc0, bi0d=bi0, b1d=b1c)

    xbf = (
        np.ascontiguousarray(x.reshape(N_CORES, BS, F, D).transpose(0, 2, 1, 3))
        .astype(BF16)
        .reshape(N_CORES, F, T)
        .astype(np.float32)
    )

    in_maps = []
    for ci in range(N_CORES):
        xc = xbf[ci]
        z0lin = (ZSCALE * xc[I] * xc[J]).astype(FP8)
        z0dc = np.ascontiguousarray(
            z0lin.reshape(NK0, 2, 128, NPAIR, 2, PAIR // 2).transpose(3, 4, 2, 0, 1, 5)
        )
        rows = xc[(2 * np.arange(16))[:, None] + (pidx // 64)[None, :]]
        xe16 = np.ascontiguousarray(
            rows[:ND16].reshape(ND16, 128, NPAIR, PAIR).transpose(2, 1, 0, 3)
        ).astype(BF16)
        xe8 = np.ascontiguousarray(
            rows[ND16 : ND16 + NC8].reshape(NC8, 128, NPAIR, PAIR).transpose(2, 1, 0, 3)
        ).astype(FP8)
        m = dict(shared)
        m["z0d"] = z0dc
        m["xe16d"] = xe16
        m["xe8d"] = xe8
        m["xt2d"] = np.ascontiguousarray(np.tile(xc, (4, 1))).astype(BF16)
        in_maps.append(m)
    return in_maps


def kernel(cin_inputs, w0, b0, w1, b1, _trace=False):
    from concourse.bass_utils import run_bass_kernel_spmd

    in_maps = _host_prep(cin_inputs, w0, b0, w1, b1)
    nc = _get_nc()
    res = run_bass_kernel_spmd(nc, in_maps, core_ids=list(range(N_CORES)), trace=_trace)
    outs = []
    for r in res.results:
        o = np.concatenate([r["out0"], r["out1"]], axis=0).T
        outs.append(o)
    full = np.concatenate(outs, axis=0).astype(np.float32)
    if _trace:
        return full, res
    return full
